# revision 72
# baseline (speedup 1.0000x reference)
"""Trainium2 Bass kernel: attention block (QKV + RoPE + ALiBi attention + proj).

Problem shapes: x [2, 2048, 1024], 16 heads x 64 dim, f32 I/O.
Sharding: batch (2) x head-groups (4 heads/core) = 8 cores. Two
AllToAlls (one per slot pair) exchange per-head outputs; each core
keeps its query-quarter and runs the output projection.

v3 schedule: one continuous 128x128-mode PE stream. A short preamble
computes just enough QKV/V (k01, q01 half-0, v chunks 0-3) for
attention to start ~30us in; the rest of QKV/V runs as PE fillers
inside attn0's j-loop so the Scalar exp stream (the attention pacer)
fully overlaps the projection matmuls. RoPE tables are shared q/k bf16
(SCALE folded into the exp activation's scale).

Softmax normalization is deferred past the collective: the AV matmul's
ones-column gives per-query denominators as a 65th row; y ships
UNNORMALIZED (65-row blocks, den included) through the AllToAll (the
260-row payload is duplicated into both batch-halves' blocks since
4-core groups are unsupported); the receiver gathers its batch's rows
by indirect DMA, reciprocates the den rows, broadcasts them with one
K=4 PE matmul per ci-tile and normalizes. This removes the per-slot
reciprocal/broadcast/mul chain (and all Scalar den copies) from the
attention critical path.

The static Tile scheduler models collectives optimistically, so every
a2a-gated op is pinned (add_dep_helper) behind attention's last use of
its engine queue - otherwise a stalled gather/cast blocks the whole
Vector/GpSimd queue mid-attention. The pair-1 AllToAll is overlapped
by the pair-0 receive chain + proj pass 1.

Dataflow (transposed on-chip):
  xT [C, N] --QKV--> qT/kT [D, N] (RoPE'd, d pairwise-interleaved) and v [N, D]
  sT = bd(kT).T @ qd  [j, i] -> p = exp(SCALE*sT) * ebias  (no max-sub)
  oT = v_aug.T @ p    [d+1, i] (ones column gives softmax denominators)
  evac bf16 [65, N] -> AllToAll -> gather -> recip+bcast+mul -> proj.
ALiBi is applied multiplicatively (exp(slope*min(j-i,0)) Toeplitz tables);
far-past tiles below ~1e-3 relative mass are skipped per-slot (SLOT_CUT).
"""
import sys
if "/opt/trn_rl_repo" not in sys.path:
    sys.path.insert(0, "/opt/trn_rl_repo")

import numpy as np
import ml_dtypes

import concourse.bass as bass
import concourse.mybir as mybir
import concourse.tile as tile
from concourse import bacc
from concourse.bass_utils import run_bass_kernel_spmd
from concourse.tile_rust import add_dep_helper

BF = mybir.dt.bfloat16
F32 = mybir.dt.float32
NPBF = ml_dtypes.bfloat16

B, N, C = 2, 2048, 1024
H, D = 16, 64
G = 4                       # heads per core
N_CORES = 8
MAX_BIAS = 8.0
SCALE = D ** -0.5
RG8 = [[0, 1, 2, 3, 4, 5, 6, 7]]

# head dealing (snake by per-head band cost) and per-slot band cutoffs
GROUP_HEADS = [[15, 8, 7, 0], [14, 9, 6, 1], [13, 10, 5, 2], [12, 11, 4, 3]]
# dropped-band mass at cut=128 is <=0.06% of the softmax denominator even
# for the shallowest slope (future keys carry full weight in this model)
SLOT_CUT = [128, 128, 128, 128]

NJ = N // 128               # 16 j-chunks
NI = N // 512               # 4 i-slices
EB_W = 1408                 # ebias table width; u = (i - j) + EB_OFF
EB_OFF = 511                # covers i0-j0 in (-512, 383+512)

# d-permutation inside each head: [0, 32, 1, 33, ...] so rotate_half becomes
# an even/odd partition swap (StreamShuffle-able within 32-row quadrants).
D_PERM = [x for i in range(32) for x in (i, i + 32)]
SHUF_MASK = [x for i in range(16) for x in (2 * i + 1, 2 * i)]


def kept_j_chunks(cut, i0):
    return [j for j in range(NJ) if i0 - (j * 128 + 127) <= cut]


def build_program(dbg=False):
    nc = bacc.Bacc("TRN2", target_bir_lowering=False, debug=False,
                   num_devices=N_CORES)
    dbg_outs = {}

    def dbg_tensor(name, shape, dt_=F32):
        dbg_outs[name] = nc.dram_tensor(name, shape, dt_, kind="ExternalOutput")
        return dbg_outs[name]

    xT = nc.dram_tensor("xT", [C, N], BF, kind="ExternalInput")
    wT = nc.dram_tensor("wT", [C, 768], BF, kind="ExternalInput")
    ct = nc.dram_tensor("ct", [128, N], BF, kind="ExternalInput")
    st = nc.dram_tensor("st", [128, N], BF, kind="ExternalInput")
    ebias = nc.dram_tensor("ebias", [G, 128, EB_W], BF, kind="ExternalInput")
    pwT = nc.dram_tensor("pwT", [C, C], BF, kind="ExternalInput")
    pb = nc.dram_tensor("pb", [128, 8], F32, kind="ExternalInput")
    qoff = nc.dram_tensor("qoff", [128, 3], mybir.dt.uint32,
                          kind="ExternalInput")
    wselt = nc.dram_tensor("wselt", [4, 256], BF, kind="ExternalInput")
    out = nc.dram_tensor("out", [C, 512], F32, kind="ExternalOutput")

    with tile.TileContext(nc) as tc:
        with tc.tile_pool(name="persist", bufs=1) as pp, \
             tc.tile_pool(name="work", bufs=1) as wp, \
             tc.tile_pool(name="psum", bufs=1, space="PSUM") as psp, \
             tc.tile_pool(name="dram", bufs=1, space="DRAM") as dp:

            # ---- persistent tiles ----
            qkp = [pp.tile([128, N], BF, name=f"qk{m}") for m in range(4)]
            vts = [pp.tile([128, 2 * G * 65], BF, name=f"vt{j}")
                   for j in range(NJ // 2)]

            def v_ap(j, slot):
                base = (j % 2) * G * 65 + slot * 65
                return vts[j // 2][:, base:base + 65]

            # unnormalized y + den per slot: [65 rows, N]
            ysb = [pp.tile([65, N], BF, name=f"ysb{s}") for s in range(4)]
            pbt = pp.tile([128, 8], F32, name="pbt")
            qot = pp.tile([128, 3], mybir.dt.uint32, name="qot")
            ones64 = pp.tile([1, 64], BF, name="ones64")
            nc.vector.memset(ones64[:], 1.0)
            # norm-broadcast selector weights: wsel[:, 128*hh:] [4, 128]:
            # cols 0:64 pick rec row 2hh, cols 64:128 pick row 2hh+1
            wsel = pp.tile([4, 256], BF, name="wsel")
            nc.sync.dma_start(wsel[:], wselt.ap()[:, :])

            # ---- input DMAs (N-half order: everything attention half-0
            # needs lands first so the interleaved front can start early) ----
            ph1 = tc.tile_pool(name="ph1", bufs=1)
            p1 = ph1.__enter__()
            # single big tiles + one strided descriptor per transfer: DMA
            # issue on the Sync queue costs ~0.6-1us per descriptor, which
            # otherwise gates the whole front
            xbig = p1.tile([128, 8 * N], BF, name="xbig")
            wbig = p1.tile([128, 8 * 768], BF, name="wbig")
            xts = [xbig[:, ci * N:(ci + 1) * N] for ci in range(8)]
            wts = [wbig[:, ci * 768:(ci + 1) * 768] for ci in range(8)]
            x3d = xbig[:].rearrange("p (c w) -> p c w", w=N)
            xs3d = xT.ap().rearrange("(c p) w -> p c w", p=128)
            nc.sync.dma_start(x3d[:, :, 0:1024], xs3d[:, :, 0:1024])
            nc.sync.dma_start(
                wbig[:].rearrange("p (c w) -> p c w", w=768),
                wT.ap().rearrange("(c p) w -> p c w", p=128))
            ctt = p1.tile([128, N], BF, name="ctt")
            nc.sync.dma_start(ctt[:, 0:1024], ct.ap()[:, 0:1024])
            stt = p1.tile([128, N], BF, name="stt")
            nc.sync.dma_start(stt[:, 0:1024], st.ap()[:, 0:1024])
            ebs = [pp.tile([128, EB_W], BF, name=f"eb{s}")
                   for s in range(G)]
            nc.sync.dma_start(ebs[0][:], ebias.ap()[0, :, :])
            nc.sync.dma_start(ebs[1][:], ebias.ap()[1, :, :])
            nc.sync.dma_start(x3d[:, :, 1024:2048], xs3d[:, :, 1024:2048])
            nc.sync.dma_start(ctt[:, 1024:2048], ct.ap()[:, 1024:2048])
            nc.sync.dma_start(stt[:, 1024:2048], st.ap()[:, 1024:2048])
            nc.sync.dma_start(pbt[:], pb.ap()[:, :])
            nc.sync.dma_start(qot[:], qoff.ap()[:, :])
            warm = wp.tile([1, 8], F32, name="warm", tag="warm", bufs=1)
            nc.vector.memset(warm[:], 1.0)
            nc.scalar.activation(warm[:], warm[:],
                                 mybir.ActivationFunctionType.Exp)
            # dense matmul burst while input DMAs run: pushes the PE HAM
            # clock-gate to 8/8 before the real matmuls begin
            wmv = wp.tile([1, 512], BF, name="wmv", tag="wmv", bufs=1)
            nc.vector.memset(wmv[:], 1.0)
            for w_ in range(18):
                Rw = psp.tile([64, 512], F32, name="Rw", tag="s", bufs=2,
                              padded_shape=[128, 512])
                nc.tensor.matmul(Rw[:], ones64[:], wmv[:], start=True,
                                 stop=True)
                if w_ >= 16:
                    nc.vector.tensor_copy(warm[:], Rw[0:1, 0:8])
            pwts = [pp.tile([128, C], BF, name=f"pwt{ci}")
                    for ci in range(8)]
            ot_acc = [pp.tile([128, 512], F32, name=f"oacc{co}")
                      for co in range(8)]

            apool = [None]

            def late_dmas():
                for s in range(2, G):
                    nc.sync.dma_start(ebs[s][:], ebias.ap()[s, :, :])
                for ci in range(8):
                    nc.sync.dma_start(pwts[ci][:],
                                      pwT.ap()[ci * 128:(ci + 1) * 128, :])

            # ---- building blocks ----
            def qkv_ip(m, ip):
                # one [128, 1024] psum group of q/k projection + RoPE
                # wT cols: q01 | q23 | k01 | k23 | v
                sl = slice(ip * 1024, (ip + 1) * 1024)
                ps = psp.tile([128, 1024], F32, name="qkvps", tag="s", bufs=2)
                for ci in range(8):
                    for hh in range(2):
                        hs = slice((2 * ip + hh) * 512,
                                   (2 * ip + hh + 1) * 512)
                        nc.tensor.matmul(
                            ps[:, hh * 512:(hh + 1) * 512],
                            wts[ci][:, m * 128:(m + 1) * 128],
                            xts[ci][:, hs],
                            start=(ci == 0), stop=(ci == 7))
                rot = wp.tile([128, 1024], F32, name="rot", tag="rot", bufs=1)
                nc.vector.stream_shuffle(rot[:], ps[:], SHUF_MASK)
                t1 = wp.tile([128, 1024], BF, name="ropet1", tag="ropet1",
                             bufs=2)
                nc.vector.tensor_mul(t1[:], rot[:], stt[:, sl])
                t2 = wp.tile([128, 1024], BF, name="ropet2", tag="ropet2",
                             bufs=2)
                nc.vector.tensor_mul(t2[:], ps[:], ctt[:, sl])
                nc.vector.tensor_add(qkp[m][:, sl], t2[:], t1[:])

            def qkv_hh(m, ip, hh):
                # half-granularity qkv_ip: one [128, 512] psum group + RoPE,
                # so the preamble's first k/q columns are ready earlier
                hs = slice((2 * ip + hh) * 512, (2 * ip + hh + 1) * 512)
                ps = psp.tile([128, 512], F32, name="qkvph", tag="s", bufs=2)
                for ci in range(8):
                    nc.tensor.matmul(ps[:], wts[ci][:, m * 128:(m + 1) * 128],
                                     xts[ci][:, hs],
                                     start=(ci == 0), stop=(ci == 7))
                rot = wp.tile([128, 512], F32, name="roth", tag="rot", bufs=1)
                nc.vector.stream_shuffle(rot[:], ps[:], SHUF_MASK)
                t1 = wp.tile([128, 512], BF, name="ropeh1", tag="ropet1",
                             bufs=2)
                nc.vector.tensor_mul(t1[:], rot[:], stt[:, hs])
                t2 = wp.tile([128, 512], BF, name="ropeh2", tag="ropet2",
                             bufs=2)
                nc.vector.tensor_mul(t2[:], ps[:], ctt[:, hs])
                nc.vector.tensor_add(qkp[m][:, hs], t2[:], t1[:])

            def vdir_jp(jp):
                # v for j-chunks 2jp, 2jp+1 (direct, non-transposed).
                # Shares the 's' psum ring (the oT banks are held across
                # whole attention halves, which would deadlock fillers).
                pv = psp.tile([128, 512], F32, name="vps",
                              tag="s", bufs=2)
                for hh in range(2):
                    j = 2 * jp + hh
                    for ci in range(8):
                        nc.tensor.matmul(
                            pv[:, hh * 256:(hh + 1) * 256],
                            xts[ci][:, j * 128:(j + 1) * 128],
                            wts[ci][:, 512:768],
                            start=(ci == 0), stop=(ci == 7))
                vt_v = vts[jp][:].rearrange("p (a h e) -> p a h e", a=2, e=65)
                nc.vector.tensor_copy(
                    vt_v[:, :, :, 0:64],
                    pv[:].rearrange("p (a h e) -> p a h e", a=2, e=64))
                nc.vector.memset(vt_v[:, :, :, 64:65], 1.0)

            # ---- a2a + receiver-side normalization + projection ----
            ag_outs = []
            recs = [None, None]
            ytf = [None] * 8

            # 65-row blocks (64 y rows + 1 den row) per query-quarter;
            # block 65*(4b'+q) goes to rank 4b'+q (duplicated into both
            # batch halves: 4-core groups are unsupported). Staging is
            # split per i-half so half-0 ships while attention half-1
            # computes; the collective triggers right after the last
            # half-1 staging DMA.
            ag_ins = [dp.tile([520, 1024], BF, name=f"ag_in{pr}")
                      for pr in range(2)]
            ag_outs.extend(dp.tile([520, 1024], BF, name=f"ag_out{pr}")
                           for pr in range(2))

            def stage_half(pr, half):
                for t in range(2):
                    slot = 2 * pr + t
                    c0 = t * 512
                    src = ysb[slot][:, 1024 * half:1024 * (half + 1)
                                    ].rearrange("e (q w) -> e q w", w=512)
                    for bb in range(2):
                        r0 = 260 * bb + 130 * half
                        dst = ag_ins[pr][r0:r0 + 130,
                                         c0:c0 + 512].rearrange(
                            "(q e) w -> e q w", e=65)
                        nc.sync.dma_start(dst, src)

            def emit_a2a_pair(pr):
                stage_half(pr, 1)
                with nc.named_scope(f"a2a{pr}"):
                    cc = nc.gpsimd.collective_compute(
                        "AllToAll", mybir.AluOpType.bypass,
                        replica_groups=RG8,
                        ins=[ag_ins[pr].opt()], outs=[ag_outs[pr].opt()])
                return cc

            def emit_ytf_pair(pr, after=(), gather_after=()):
                # gather my batch's y rows + den rows, reciprocate,
                # broadcast (K=4 matmul) and normalize for the proj matmuls.
                # `after`/`gather_after`: instructions the head Vector/GpSimd
                # ops must be ordered behind (the static scheduler models
                # collectives optimistically; without the pins it slots these
                # a2a-gated ops ahead of attention work on the same engine
                # queues). The gathers may pin earlier: GpSimd is idle during
                # attention so a stalled gather blocks nothing.
                def pin(bi, deps):
                    for a in deps:
                        add_dep_helper(bi.ins, a.ins, reason="post-attn pin")
                    return bi
                ag_out = ag_outs[pr]
                dent = apool[0].tile([4, 1024], BF, name="dent",
                                     tag=f"dent{pr}", bufs=1)
                pin(nc.gpsimd.indirect_dma_start(
                    out=dent[:], out_offset=None,
                    in_=ag_out[:],
                    in_offset=bass.IndirectOffsetOnAxis(
                        ap=qot[:, 2:3], axis=0)), gather_after)
                rec = apool[0].tile([4, 1024], BF, name="rec",
                                    tag=f"rec{pr}", bufs=1)
                with nc.allow_low_precision("den/rec already ship as bf16"):
                    pin(nc.vector.reciprocal(rec[:], dent[:]), after)
                recs[pr] = rec
                for hh in range(2):
                    yr = pp.tile([128, 1024], BF, name=f"ytr{pr}_{hh}")
                    pin(nc.gpsimd.indirect_dma_start(
                        out=yr[:], out_offset=None,
                        in_=ag_out[:],
                        in_offset=bass.IndirectOffsetOnAxis(
                            ap=qot[:, hh:hh + 1], axis=0)), gather_after)
                    for t in range(2):
                        slot = 2 * pr + t
                        ytf[2 * slot + hh] = yr[:, 512 * t:512 * (t + 1)]

            def norm_ytf(ci, after=()):
                # ytf[ci] *= bcast(rec rows) ; ci = 2*slot + hh
                pr, t, hh = ci // 4, (ci // 2) % 2, ci % 2
                bc = psp.tile([128, 512], F32, name="bc", tag="s", bufs=2)
                mm = nc.tensor.matmul(bc[:],
                                      wsel[:, 128 * hh:128 * (hh + 1)],
                                      recs[pr][:, 512 * t:512 * (t + 1)],
                                      start=True, stop=True)
                for a in after:
                    add_dep_helper(mm.ins, a.ins, reason="post-attn pin")
                return nc.vector.tensor_mul(ytf[ci][:], ytf[ci][:], bc[:])

            def proj_pass1(co):
                # pair-0 contribution (ci 0..3) -> SBUF accumulator
                pj = psp.tile([128, 512], F32, name="pj", tag="s", bufs=2)
                for ci in range(4):
                    mm = nc.tensor.matmul(pj[:],
                                          pwts[ci][:, co * 128:(co + 1) * 128],
                                          ytf[ci][:], start=(ci == 0),
                                          stop=(ci == 3))
                cp = nc.scalar.copy(ot_acc[co][:], pj[:])
                return mm, cp

            def proj_pass2(co):
                pj = psp.tile([128, 512], F32, name="pj", tag="s", bufs=2)
                for ci in range(4, 8):
                    nc.tensor.matmul(pj[:],
                                     pwts[ci][:, co * 128:(co + 1) * 128],
                                     ytf[ci][:], start=(ci == 4),
                                     stop=(ci == 7))
                ot = apool[0].tile([128, 512], F32, name="ot", tag="ot", bufs=2)
                nc.vector.scalar_tensor_tensor(
                    ot[:], pj[:], pbt[:, co:co + 1], ot_acc[co][:],
                    mybir.AluOpType.add, mybir.AluOpType.add)
                nc.sync.dma_start(out.ap()[co * 128:(co + 1) * 128, :], ot[:])

            def attn_pair(pr, fillers):
                """Attention for slot pair (2pr, 2pr+1). QK runs as two
                concurrent 64-row PE tiles (slot A on partitions 0-63, B on
                64-127). Tiles are (j-chunk, isl); processed in batches of
                two. isl-halves (0,1)/(2,3) run sequentially so both slots'
                oT accumulators fit in PSUM. After each half the oT psum
                (64 y rows + den row) is evacuated with a single bf16 cast
                per slot into ysb. fillers: (after_tile_count, fn)."""
                cut = SLOT_CUT[2 * pr]
                sA, sB = 2 * pr, 2 * pr + 1
                kq = qkp[pr]
                kk = qkp[2 + pr]
                fi = 0
                nt = 0
                with nc.named_scope(f"attn{pr}"):
                    for half in range(2):
                        isls = (0, 1) if half == 0 else (2, 3)
                        kept = {isl: kept_j_chunks(cut, isl * 512)
                                for isl in isls}
                        oTs = [psp.tile([65, 1024], F32, name=f"oTp{i}",
                                        tag=f"oTp{i}", bufs=1,
                                        padded_shape=[128, 1024])
                               for i in range(2)]
                        tl = [(j, isl) for j in range(NJ) for isl in isls
                              if j in kept[isl]]

                        def emit_av_batch(batch):
                            # slot-major so consecutive matmuls share the
                            # same v weight tile (no alternating LDWEIGHTS)
                            for t in range(2):
                                slot = sA if t == 0 else sB
                                for (j, isl, p) in batch:
                                    c0 = (isl % 2) * 512
                                    nc.tensor.matmul(
                                        oTs[t][:, c0:c0 + 512],
                                        v_ap(j, slot),
                                        p[:, t * 512:(t + 1) * 512],
                                        start=(j == kept[isl][0]),
                                        stop=(j == kept[isl][-1]))

                        pend = []
                        for ti in range(0, len(tl), 2):
                            cur = []
                            for (j, isl) in tl[ti:ti + 2]:
                                while (fi < len(fillers)
                                       and fillers[fi][0] <= nt):
                                    fillers[fi][1]()
                                    fi += 1
                                nt += 1
                                j0 = j * 128
                                i0 = isl * 512
                                s = psp.tile([128, 1024], F32, name="s",
                                             tag="s", bufs=2)
                                nc.tensor.matmul(
                                    s[:, 0:512], kk[0:64, j0:j0 + 128],
                                    kq[0:64, i0:i0 + 512],
                                    start=True, stop=True,
                                    tile_position=(0, 0))
                                nc.tensor.matmul(
                                    s[:, 512:1024], kk[64:128, j0:j0 + 128],
                                    kq[64:128, i0:i0 + 512],
                                    start=True, stop=True,
                                    tile_position=(64, 0))
                                p = apool[0].tile([128, 1024], BF, name="p",
                                                  tag="p", bufs=6)
                                nc.scalar.activation(
                                    p[:], s[:],
                                    mybir.ActivationFunctionType.Exp,
                                    scale=SCALE)
                                if j0 - i0 < 512:
                                    off = i0 - j0 + EB_OFF
                                    nc.vector.tensor_mul(
                                        p[:, 0:512], p[:, 0:512],
                                        ebs[sA][:, off:off + 512])
                                    nc.vector.tensor_mul(
                                        p[:, 512:1024], p[:, 512:1024],
                                        ebs[sB][:, off:off + 512])
                                cur.append((j, isl, p))
                            emit_av_batch(pend)
                            pend = cur
                        emit_av_batch(pend)
                        # evacuate oT psum (y + den) -> SBUF bf16, one cast
                        # per slot; frees the banks for the next half/pair
                        i0 = half * 1024
                        for t in range(2):
                            slot = sA if t == 0 else sB
                            last_evac = nc.vector.tensor_copy(
                                ysb[slot][:, i0:i0 + 1024], oTs[t][0:65, :])
                        if half == 0:
                            half0_evac = last_evac
                    while fi < len(fillers):
                        fillers[fi][1]()
                        fi += 1
                return half0_evac, last_evac

            # ================= emission schedule =================
            # Interleaved front: the preamble computes just enough QKV/V
            # (k01, q01 half-0, v chunks 0-3) for attention to start; the
            # rest of QKV and V are PE fillers inside attn0's j-loop so
            # the Scalar exp stream overlaps the projection matmuls.
            appool = tc.tile_pool(name="attn", bufs=1)
            apool[0] = appool.__enter__()
            sc = nc.enter_named_scope("qkv0", False)
            qkv_hh(2, 0, 0)
            qkv_hh(0, 0, 0)
            qkv_hh(2, 0, 1)
            vdir_jp(0)
            qkv_hh(0, 0, 1)
            vdir_jp(1)
            vdir_jp(2)
            nc.leave_named_scope("qkv0", sc[0], False)

            _ = attn_pair(0, [
                (5, lambda: vdir_jp(3)),
                (8, lambda: qkv_ip(2, 1)),
                (11, lambda: vdir_jp(4)),
                (14, lambda: vdir_jp(5)),
                (17, lambda: vdir_jp(6)),
                (20, late_dmas),
                (21, lambda: vdir_jp(7)),
                (24, lambda: qkv_ip(0, 1)),
                (29, lambda: stage_half(0, 0)),
                (30, lambda: qkv_ip(3, 0)),
                (34, lambda: qkv_ip(3, 1)),
                (38, lambda: qkv_ip(1, 0)),
            ])
            emit_a2a_pair(0)
            ev_half0, ev_last = attn_pair(1, [
                (6, lambda: qkv_ip(1, 1)),
                (30, lambda: stage_half(1, 0)),
            ])
            emit_a2a_pair(1)
            emit_ytf_pair(0, after=(ev_last,), gather_after=(ev_half0,))
            for ci in range(4):
                nm = norm_ytf(ci)
            for co in range(8):
                p1mm, p1cp = proj_pass1(co)
            emit_ytf_pair(1, after=(nm, p1cp), gather_after=(nm, p1cp))
            for ci in range(4, 8):
                norm_ytf(ci, after=(p1mm,))

            if dbg:
                for m in range(4):
                    t = dbg_tensor(f"dbg_qk{m}", [128, N], BF)
                    nc.sync.dma_start(t.ap()[:, :], qkp[m][:])
                t = dbg_tensor("dbg_vt0", [128, 2 * G * 65], BF)
                nc.sync.dma_start(t.ap()[:, :], vts[0][:])
                for s_ in range(4):
                    t = dbg_tensor(f"dbg_ysb{s_}", [65, N], BF)
                    nc.sync.dma_start(t.ap()[:, :], ysb[s_][:])
                for hh in range(2):
                    t = dbg_tensor(f"dbg_ytf{hh}", [128, 512], BF)
                    nc.sync.dma_start(t.ap()[:, :], ytf[hh][:])
                t = dbg_tensor("dbg_oacc0", [128, 512], F32)
                nc.sync.dma_start(t.ap()[:, :], ot_acc[0][:])

            sc = nc.enter_named_scope("proj", False)
            for co in range(8):
                proj_pass2(co)
            nc.leave_named_scope("proj", sc[0], False)
            appool.__exit__(None, None, None)
            ph1.__exit__(None, None, None)

    nc.compile()
    return nc


def prep_inputs(x, qkv_w, proj_w, proj_b, slopes):
    """Build the 8 per-core input maps (all host-side numpy)."""
    x = np.asarray(x, np.float32)
    qkv_w = np.asarray(qkv_w, np.float32)
    proj_w = np.asarray(proj_w, np.float32)
    proj_b = np.asarray(proj_b, np.float32)
    slopes = np.asarray(slopes, np.float32)

    # RoPE tables (transposed [d, n], d pairwise-interleaved, x2 head copies)
    inv = 1.0 / (10000.0 ** (np.arange(0, D, 2, dtype=np.float32) / D))
    fr = np.arange(N, dtype=np.float32)[:, None] * inv[None, :]   # [N, 32]
    sin_t, cos_t = np.sin(fr), np.cos(fr)
    ct64 = np.empty((64, N), np.float32)
    st64 = np.empty((64, N), np.float32)
    ct64[0::2] = cos_t.T
    ct64[1::2] = cos_t.T
    st64[0::2] = -sin_t.T
    st64[1::2] = sin_t.T
    ct = np.ascontiguousarray(np.vstack([ct64, ct64])).astype(NPBF)
    st = np.ascontiguousarray(np.vstack([st64, st64])).astype(NPBF)

    pos_p = np.arange(128, dtype=np.float64)[:, None]
    t_off = np.arange(EB_W, dtype=np.float64)[None, :] - EB_OFF
    dmin = np.minimum(pos_p - t_off, 0.0)  # j - i clipped

    in_maps = []
    for core in range(N_CORES):
        b = core // 4
        g = core % 4
        heads = GROUP_HEADS[g]
        rows = []
        for kind in range(2):  # q, k (d-permuted)
            for h in heads:
                base = kind * C + h * D
                rows.extend(base + p for p in D_PERM)
        for h in heads:        # v (natural d order)
            rows.extend(2 * C + h * D + d for d in range(D))
        wT_c = np.ascontiguousarray(qkv_w[rows, :].T)      # [1024, 768]

        eb_c = np.empty((G, 128, EB_W), np.float32)
        for s, h in enumerate(heads):
            eb_c[s] = np.exp(float(slopes[h]) * MAX_BIAS * dmin)

        # pwT rows (ci) ordered as the a2a outputs: ci = 2*slot + hh rows =
        # [rank 2hh (slot's head, 64 d), rank 2hh+1 (...)]
        pwT_c = np.empty((C, C), np.float32)
        for s in range(4):
            for rank in range(4):
                hh = GROUP_HEADS[rank][s]
                r = 256 * s + 64 * rank
                pwT_c[r:r + 64, :] = proj_w[:, hh * D:(hh + 1) * D].T
        pb_c = np.ascontiguousarray(proj_b.reshape(8, 128).T)

        # gather row offsets into ag_out [520, 1024]: my batch's rows start
        # at 260*b; rank r's block at 65*r, 64 y rows + 1 den row each.
        qoff_c = np.zeros((128, 3), np.uint32)
        p_ = np.arange(128)
        for hh in range(2):
            r = 2 * hh + p_ // 64
            qoff_c[:, hh] = 260 * b + 65 * r + p_ % 64
        qoff_c[0:4, 2] = 260 * b + 65 * np.arange(4) + 64

        wsel_c = np.zeros((4, 256), np.float32)
        for hh in range(2):
            wsel_c[2 * hh, 128 * hh:128 * hh + 64] = 1.0
            wsel_c[2 * hh + 1, 128 * hh + 64:128 * hh + 128] = 1.0

        in_maps.append({
            "xT": np.ascontiguousarray(x[b].T).astype(NPBF),
            "wT": wT_c.astype(NPBF),
            "ct": ct, "st": st,
            "ebias": eb_c.astype(NPBF),
            "pwT": pwT_c.astype(NPBF),
            "pb": pb_c,
            "qoff": qoff_c,
            "wselt": wsel_c.astype(NPBF),
        })
    return in_maps


_NC = None


def _get_nc():
    global _NC
    if _NC is None:
        _NC = build_program()
    return _NC


def run(inputs, trace=False, **kw):
    nc = _get_nc()
    in_maps = prep_inputs(**inputs)
    res = run_bass_kernel_spmd(nc, in_maps, core_ids=list(range(N_CORES)),
                               trace=trace, **kw)
    out = np.empty((B, N, C), np.float32)
    for core in range(N_CORES):
        b, g = core // 4, core % 4
        out[b, g * 512:(g + 1) * 512, :] = res.results[core]["out"].T
    return out, res


def kernel(**inputs) -> np.ndarray:
    out, _ = run(inputs, trace=False)
    return out


# revision 73
# speedup vs baseline: 37.8082x; 37.8082x over previous
"""Trainium2 Bass kernel: attention block (QKV + RoPE + ALiBi attention + proj).

Problem shapes: x [2, 2048, 1024], 16 heads x 64 dim, f32 I/O.
Sharding: batch (2) x head-groups (4 heads/core) = 8 cores. Two
AllToAlls (one per slot pair) exchange per-head outputs; each core
keeps its query-quarter and runs the output projection.

v3 schedule: one continuous 128x128-mode PE stream. A short preamble
computes just enough QKV/V (k01, q01 half-0, v chunks 0-3) for
attention to start ~30us in; the rest of QKV/V runs as PE fillers
inside attn0's j-loop so the Scalar exp stream (the attention pacer)
fully overlaps the projection matmuls. RoPE tables are shared q/k bf16
(SCALE folded into the exp activation's scale).

Softmax normalization is deferred past the collective: the AV matmul's
ones-column gives per-query denominators as a 65th row; y ships
UNNORMALIZED (65-row blocks, den included) through the AllToAll (the
260-row payload is duplicated into both batch-halves' blocks since
4-core groups are unsupported); the receiver gathers its batch's rows
by indirect DMA, reciprocates the den rows, broadcasts them with one
K=4 PE matmul per ci-tile and normalizes. This removes the per-slot
reciprocal/broadcast/mul chain (and all Scalar den copies) from the
attention critical path.

The static Tile scheduler models collectives optimistically, so every
a2a-gated op is pinned (add_dep_helper) behind attention's last use of
its engine queue - otherwise a stalled gather/cast blocks the whole
Vector/GpSimd queue mid-attention. The pair-1 AllToAll is overlapped
by the pair-0 receive chain + proj pass 1.

Dataflow (transposed on-chip):
  xT [C, N] --QKV--> qT/kT [D, N] (RoPE'd, d pairwise-interleaved) and v [N, D]
  sT = bd(kT).T @ qd  [j, i] -> p = exp(SCALE*sT) * ebias  (no max-sub)
  oT = v_aug.T @ p    [d+1, i] (ones column gives softmax denominators)
  evac bf16 [65, N] -> AllToAll -> gather -> recip+bcast+mul -> proj.
ALiBi is applied multiplicatively (exp(slope*min(j-i,0)) Toeplitz tables);
far-past tiles below ~1e-3 relative mass are skipped per-slot (SLOT_CUT).
"""
import sys
if "/opt/trn_rl_repo" not in sys.path:
    sys.path.insert(0, "/opt/trn_rl_repo")

import numpy as np
import ml_dtypes

import concourse.bass as bass
import concourse.mybir as mybir
import concourse.tile as tile
from concourse import bacc
from concourse.bass_utils import run_bass_kernel_spmd
from concourse.tile_rust import add_dep_helper

BF = mybir.dt.bfloat16
F32 = mybir.dt.float32
NPBF = ml_dtypes.bfloat16

B, N, C = 2, 2048, 1024
H, D = 16, 64
G = 4                       # heads per core
N_CORES = 8
MAX_BIAS = 8.0
SCALE = D ** -0.5
RG8 = [[0, 1, 2, 3, 4, 5, 6, 7]]

# head dealing (snake by per-head band cost) and per-slot band cutoffs
GROUP_HEADS = [[15, 8, 7, 0], [14, 9, 6, 1], [13, 10, 5, 2], [12, 11, 4, 3]]
# dropped-band mass at cut=128 is <=0.06% of the softmax denominator even
# for the shallowest slope (future keys carry full weight in this model)
SLOT_CUT = [128, 128, 128, 128]

NJ = N // 128               # 16 j-chunks
NI = N // 512               # 4 i-slices
EB_W = 1408                 # ebias table width; u = (i - j) + EB_OFF
EB_OFF = 511                # covers i0-j0 in (-512, 383+512)

# d-permutation inside each head: [0, 32, 1, 33, ...] so rotate_half becomes
# an even/odd partition swap (StreamShuffle-able within 32-row quadrants).
D_PERM = [x for i in range(32) for x in (i, i + 32)]
SHUF_MASK = [x for i in range(16) for x in (2 * i + 1, 2 * i)]


def kept_j_chunks(cut, i0):
    return [j for j in range(NJ) if i0 - (j * 128 + 127) <= cut]


def build_program(dbg=False):
    nc = bacc.Bacc("TRN2", target_bir_lowering=False, debug=False,
                   num_devices=N_CORES)
    dbg_outs = {}

    def dbg_tensor(name, shape, dt_=F32):
        dbg_outs[name] = nc.dram_tensor(name, shape, dt_, kind="ExternalOutput")
        return dbg_outs[name]

    xT = nc.dram_tensor("xT", [C, N], BF, kind="ExternalInput")
    wT = nc.dram_tensor("wT", [C, 768], BF, kind="ExternalInput")
    ct = nc.dram_tensor("ct", [128, N], BF, kind="ExternalInput")
    st = nc.dram_tensor("st", [128, N], BF, kind="ExternalInput")
    ebias = nc.dram_tensor("ebias", [G, 128, EB_W], BF, kind="ExternalInput")
    pwT = nc.dram_tensor("pwT", [C, C], BF, kind="ExternalInput")
    pb = nc.dram_tensor("pb", [128, 8], F32, kind="ExternalInput")
    qoff = nc.dram_tensor("qoff", [128, 3], mybir.dt.uint32,
                          kind="ExternalInput")
    wselt = nc.dram_tensor("wselt", [4, 256], BF, kind="ExternalInput")
    out = nc.dram_tensor("out", [C, 512], F32, kind="ExternalOutput")

    with tile.TileContext(nc) as tc:
        with tc.tile_pool(name="persist", bufs=1) as pp, \
             tc.tile_pool(name="work", bufs=1) as wp, \
             tc.tile_pool(name="psum", bufs=1, space="PSUM") as psp, \
             tc.tile_pool(name="dram", bufs=1, space="DRAM") as dp:

            # ---- persistent tiles ----
            qkp = [pp.tile([128, N], BF, name=f"qk{m}") for m in range(4)]
            vts = [pp.tile([128, 2 * G * 65], BF, name=f"vt{j}")
                   for j in range(NJ // 2)]

            def v_ap(j, slot):
                base = (j % 2) * G * 65 + slot * 65
                return vts[j // 2][:, base:base + 65]

            # unnormalized y + den per slot: [65 rows, N]
            ysb = [pp.tile([65, N], BF, name=f"ysb{s}") for s in range(4)]
            pbt = pp.tile([128, 8], F32, name="pbt")
            qot = pp.tile([128, 3], mybir.dt.uint32, name="qot")
            ones64 = pp.tile([1, 64], BF, name="ones64")
            nc.vector.memset(ones64[:], 1.0)
            # norm-broadcast selector weights: wsel[:, 128*hh:] [4, 128]:
            # cols 0:64 pick rec row 2hh, cols 64:128 pick row 2hh+1
            wsel = pp.tile([4, 256], BF, name="wsel")
            nc.sync.dma_start(wsel[:], wselt.ap()[:, :])

            # ---- input DMAs (N-half order: everything attention half-0
            # needs lands first so the interleaved front can start early) ----
            ph1 = tc.tile_pool(name="ph1", bufs=1)
            p1 = ph1.__enter__()
            xts, wts = [], []
            for ci in range(8):
                t = p1.tile([128, N], BF, name=f"xt{ci}")
                nc.sync.dma_start(t[:, 0:1024],
                                  xT.ap()[ci * 128:(ci + 1) * 128, 0:1024])
                xts.append(t)
                t = p1.tile([128, 768], BF, name=f"wt{ci}")
                nc.sync.dma_start(t[:], wT.ap()[ci * 128:(ci + 1) * 128, :])
                wts.append(t)
            ctt = p1.tile([128, N], BF, name="ctt")
            nc.sync.dma_start(ctt[:, 0:1024], ct.ap()[:, 0:1024])
            stt = p1.tile([128, N], BF, name="stt")
            nc.sync.dma_start(stt[:, 0:1024], st.ap()[:, 0:1024])
            ebs = [pp.tile([128, EB_W], BF, name=f"eb{s}")
                   for s in range(G)]
            nc.sync.dma_start(ebs[0][:], ebias.ap()[0, :, :])
            nc.sync.dma_start(ebs[1][:], ebias.ap()[1, :, :])
            for ci in range(8):
                nc.sync.dma_start(xts[ci][:, 1024:2048],
                                  xT.ap()[ci * 128:(ci + 1) * 128, 1024:2048])
            nc.sync.dma_start(ctt[:, 1024:2048], ct.ap()[:, 1024:2048])
            nc.sync.dma_start(stt[:, 1024:2048], st.ap()[:, 1024:2048])
            nc.sync.dma_start(pbt[:], pb.ap()[:, :])
            nc.sync.dma_start(qot[:], qoff.ap()[:, :])
            warm = wp.tile([1, 8], F32, name="warm", tag="warm", bufs=1)
            nc.vector.memset(warm[:], 1.0)
            nc.scalar.activation(warm[:], warm[:],
                                 mybir.ActivationFunctionType.Exp)
            # dense matmul burst while input DMAs run: pushes the PE HAM
            # clock-gate to 8/8 before the real matmuls begin
            wmv = wp.tile([1, 512], BF, name="wmv", tag="wmv", bufs=1)
            nc.vector.memset(wmv[:], 1.0)
            for w_ in range(18):
                Rw = psp.tile([64, 512], F32, name="Rw", tag="s", bufs=2,
                              padded_shape=[128, 512])
                nc.tensor.matmul(Rw[:], ones64[:], wmv[:], start=True,
                                 stop=True)
                if w_ >= 16:
                    nc.vector.tensor_copy(warm[:], Rw[0:1, 0:8])
            pwts = [pp.tile([128, C], BF, name=f"pwt{ci}")
                    for ci in range(8)]
            ot_acc = [pp.tile([128, 512], F32, name=f"oacc{co}")
                      for co in range(8)]

            apool = [None]

            def late_dmas():
                for s in range(2, G):
                    nc.sync.dma_start(ebs[s][:], ebias.ap()[s, :, :])
                for ci in range(8):
                    nc.sync.dma_start(pwts[ci][:],
                                      pwT.ap()[ci * 128:(ci + 1) * 128, :])

            # ---- building blocks ----
            def qkv_ip(m, ip):
                # one [128, 1024] psum group of q/k projection + RoPE
                # wT cols: q01 | q23 | k01 | k23 | v
                sl = slice(ip * 1024, (ip + 1) * 1024)
                ps = psp.tile([128, 1024], F32, name="qkvps", tag="s", bufs=2)
                for ci in range(8):
                    for hh in range(2):
                        hs = slice((2 * ip + hh) * 512,
                                   (2 * ip + hh + 1) * 512)
                        nc.tensor.matmul(
                            ps[:, hh * 512:(hh + 1) * 512],
                            wts[ci][:, m * 128:(m + 1) * 128],
                            xts[ci][:, hs],
                            start=(ci == 0), stop=(ci == 7))
                rot = wp.tile([128, 1024], F32, name="rot", tag="rot", bufs=1)
                nc.vector.stream_shuffle(rot[:], ps[:], SHUF_MASK)
                t1 = wp.tile([128, 1024], BF, name="ropet1", tag="ropet1",
                             bufs=2)
                nc.vector.tensor_mul(t1[:], rot[:], stt[:, sl])
                t2 = wp.tile([128, 1024], BF, name="ropet2", tag="ropet2",
                             bufs=2)
                nc.vector.tensor_mul(t2[:], ps[:], ctt[:, sl])
                nc.vector.tensor_add(qkp[m][:, sl], t2[:], t1[:])

            def qkv_hh(m, ip, hh):
                # half-granularity qkv_ip: one [128, 512] psum group + RoPE,
                # so the preamble's first k/q columns are ready earlier
                hs = slice((2 * ip + hh) * 512, (2 * ip + hh + 1) * 512)
                ps = psp.tile([128, 512], F32, name="qkvph", tag="s", bufs=2)
                for ci in range(8):
                    nc.tensor.matmul(ps[:], wts[ci][:, m * 128:(m + 1) * 128],
                                     xts[ci][:, hs],
                                     start=(ci == 0), stop=(ci == 7))
                rot = wp.tile([128, 512], F32, name="roth", tag="rot", bufs=1)
                nc.vector.stream_shuffle(rot[:], ps[:], SHUF_MASK)
                t1 = wp.tile([128, 512], BF, name="ropeh1", tag="ropet1",
                             bufs=2)
                nc.vector.tensor_mul(t1[:], rot[:], stt[:, hs])
                t2 = wp.tile([128, 512], BF, name="ropeh2", tag="ropet2",
                             bufs=2)
                nc.vector.tensor_mul(t2[:], ps[:], ctt[:, hs])
                nc.vector.tensor_add(qkp[m][:, hs], t2[:], t1[:])

            def vdir_jp(jp):
                # v for j-chunks 2jp, 2jp+1 (direct, non-transposed).
                # Shares the 's' psum ring (the oT banks are held across
                # whole attention halves, which would deadlock fillers).
                pv = psp.tile([128, 512], F32, name="vps",
                              tag="s", bufs=2)
                for hh in range(2):
                    j = 2 * jp + hh
                    for ci in range(8):
                        nc.tensor.matmul(
                            pv[:, hh * 256:(hh + 1) * 256],
                            xts[ci][:, j * 128:(j + 1) * 128],
                            wts[ci][:, 512:768],
                            start=(ci == 0), stop=(ci == 7))
                vt_v = vts[jp][:].rearrange("p (a h e) -> p a h e", a=2, e=65)
                nc.vector.tensor_copy(
                    vt_v[:, :, :, 0:64],
                    pv[:].rearrange("p (a h e) -> p a h e", a=2, e=64))
                nc.vector.memset(vt_v[:, :, :, 64:65], 1.0)

            # ---- a2a + receiver-side normalization + projection ----
            ag_outs = []
            recs = [None, None]
            ytf = [None] * 8

            # 65-row blocks (64 y rows + 1 den row) per query-quarter;
            # block 65*(4b'+q) goes to rank 4b'+q (duplicated into both
            # batch halves: 4-core groups are unsupported). Staging is
            # split per i-half so half-0 ships while attention half-1
            # computes; the collective triggers right after the last
            # half-1 staging DMA.
            ag_ins = [dp.tile([520, 1024], BF, name=f"ag_in{pr}")
                      for pr in range(2)]
            ag_outs.extend(dp.tile([520, 1024], BF, name=f"ag_out{pr}")
                           for pr in range(2))

            def stage_half(pr, half):
                for t in range(2):
                    slot = 2 * pr + t
                    c0 = t * 512
                    src = ysb[slot][:, 1024 * half:1024 * (half + 1)
                                    ].rearrange("e (q w) -> e q w", w=512)
                    for bb in range(2):
                        r0 = 260 * bb + 130 * half
                        dst = ag_ins[pr][r0:r0 + 130,
                                         c0:c0 + 512].rearrange(
                            "(q e) w -> e q w", e=65)
                        nc.sync.dma_start(dst, src)

            def emit_a2a_pair(pr):
                stage_half(pr, 1)
                with nc.named_scope(f"a2a{pr}"):
                    cc = nc.gpsimd.collective_compute(
                        "AllToAll", mybir.AluOpType.bypass,
                        replica_groups=RG8,
                        ins=[ag_ins[pr].opt()], outs=[ag_outs[pr].opt()])
                return cc

            def emit_ytf_pair(pr, after=(), gather_after=()):
                # gather my batch's y rows + den rows, reciprocate,
                # broadcast (K=4 matmul) and normalize for the proj matmuls.
                # `after`/`gather_after`: instructions the head Vector/GpSimd
                # ops must be ordered behind (the static scheduler models
                # collectives optimistically; without the pins it slots these
                # a2a-gated ops ahead of attention work on the same engine
                # queues). The gathers may pin earlier: GpSimd is idle during
                # attention so a stalled gather blocks nothing.
                def pin(bi, deps):
                    for a in deps:
                        add_dep_helper(bi.ins, a.ins, reason="post-attn pin")
                    return bi
                ag_out = ag_outs[pr]
                dent = apool[0].tile([4, 1024], BF, name="dent",
                                     tag=f"dent{pr}", bufs=1)
                pin(nc.gpsimd.indirect_dma_start(
                    out=dent[:], out_offset=None,
                    in_=ag_out[:],
                    in_offset=bass.IndirectOffsetOnAxis(
                        ap=qot[:, 2:3], axis=0)), gather_after)
                rec = apool[0].tile([4, 1024], BF, name="rec",
                                    tag=f"rec{pr}", bufs=1)
                with nc.allow_low_precision("den/rec already ship as bf16"):
                    pin(nc.vector.reciprocal(rec[:], dent[:]), after)
                recs[pr] = rec
                for hh in range(2):
                    yr = pp.tile([128, 1024], BF, name=f"ytr{pr}_{hh}")
                    pin(nc.gpsimd.indirect_dma_start(
                        out=yr[:], out_offset=None,
                        in_=ag_out[:],
                        in_offset=bass.IndirectOffsetOnAxis(
                            ap=qot[:, hh:hh + 1], axis=0)), gather_after)
                    for t in range(2):
                        slot = 2 * pr + t
                        ytf[2 * slot + hh] = yr[:, 512 * t:512 * (t + 1)]

            def norm_ytf(ci, after=()):
                # ytf[ci] *= bcast(rec rows) ; ci = 2*slot + hh
                pr, t, hh = ci // 4, (ci // 2) % 2, ci % 2
                bc = psp.tile([128, 512], F32, name="bc", tag="s", bufs=2)
                mm = nc.tensor.matmul(bc[:],
                                      wsel[:, 128 * hh:128 * (hh + 1)],
                                      recs[pr][:, 512 * t:512 * (t + 1)],
                                      start=True, stop=True)
                for a in after:
                    add_dep_helper(mm.ins, a.ins, reason="post-attn pin")
                return nc.vector.tensor_mul(ytf[ci][:], ytf[ci][:], bc[:])

            def proj_pass1(co):
                # pair-0 contribution (ci 0..3) -> SBUF accumulator
                pj = psp.tile([128, 512], F32, name="pj", tag="s", bufs=2)
                for ci in range(4):
                    mm = nc.tensor.matmul(pj[:],
                                          pwts[ci][:, co * 128:(co + 1) * 128],
                                          ytf[ci][:], start=(ci == 0),
                                          stop=(ci == 3))
                cp = nc.scalar.copy(ot_acc[co][:], pj[:])
                return mm, cp

            def proj_pass2(co):
                pj = psp.tile([128, 512], F32, name="pj", tag="s", bufs=2)
                for ci in range(4, 8):
                    nc.tensor.matmul(pj[:],
                                     pwts[ci][:, co * 128:(co + 1) * 128],
                                     ytf[ci][:], start=(ci == 4),
                                     stop=(ci == 7))
                ot = apool[0].tile([128, 512], F32, name="ot", tag="ot", bufs=2)
                nc.vector.scalar_tensor_tensor(
                    ot[:], pj[:], pbt[:, co:co + 1], ot_acc[co][:],
                    mybir.AluOpType.add, mybir.AluOpType.add)
                nc.sync.dma_start(out.ap()[co * 128:(co + 1) * 128, :], ot[:])

            def attn_pair(pr, fillers):
                """Attention for slot pair (2pr, 2pr+1). QK runs as two
                concurrent 64-row PE tiles (slot A on partitions 0-63, B on
                64-127). Tiles are (j-chunk, isl); processed in batches of
                two. isl-halves (0,1)/(2,3) run sequentially so both slots'
                oT accumulators fit in PSUM. After each half the oT psum
                (64 y rows + den row) is evacuated with a single bf16 cast
                per slot into ysb. fillers: (after_tile_count, fn)."""
                cut = SLOT_CUT[2 * pr]
                sA, sB = 2 * pr, 2 * pr + 1
                kq = qkp[pr]
                kk = qkp[2 + pr]
                fi = 0
                nt = 0
                with nc.named_scope(f"attn{pr}"):
                    for half in range(2):
                        isls = (0, 1) if half == 0 else (2, 3)
                        kept = {isl: kept_j_chunks(cut, isl * 512)
                                for isl in isls}
                        oTs = [psp.tile([65, 1024], F32, name=f"oTp{i}",
                                        tag=f"oTp{i}", bufs=1,
                                        padded_shape=[128, 1024])
                               for i in range(2)]
                        tl = [(j, isl) for j in range(NJ) for isl in isls
                              if j in kept[isl]]

                        def emit_av_batch(batch):
                            # slot-major so consecutive matmuls share the
                            # same v weight tile (no alternating LDWEIGHTS)
                            for t in range(2):
                                slot = sA if t == 0 else sB
                                for (j, isl, p) in batch:
                                    c0 = (isl % 2) * 512
                                    nc.tensor.matmul(
                                        oTs[t][:, c0:c0 + 512],
                                        v_ap(j, slot),
                                        p[:, t * 512:(t + 1) * 512],
                                        start=(j == kept[isl][0]),
                                        stop=(j == kept[isl][-1]))

                        pend = []
                        for ti in range(0, len(tl), 2):
                            cur = []
                            for (j, isl) in tl[ti:ti + 2]:
                                while (fi < len(fillers)
                                       and fillers[fi][0] <= nt):
                                    fillers[fi][1]()
                                    fi += 1
                                nt += 1
                                j0 = j * 128
                                i0 = isl * 512
                                s = psp.tile([128, 1024], F32, name="s",
                                             tag="s", bufs=2)
                                nc.tensor.matmul(
                                    s[:, 0:512], kk[0:64, j0:j0 + 128],
                                    kq[0:64, i0:i0 + 512],
                                    start=True, stop=True,
                                    tile_position=(0, 0))
                                nc.tensor.matmul(
                                    s[:, 512:1024], kk[64:128, j0:j0 + 128],
                                    kq[64:128, i0:i0 + 512],
                                    start=True, stop=True,
                                    tile_position=(64, 0))
                                p = apool[0].tile([128, 1024], BF, name="p",
                                                  tag="p", bufs=6)
                                nc.scalar.activation(
                                    p[:], s[:],
                                    mybir.ActivationFunctionType.Exp,
                                    scale=SCALE)
                                if j0 - i0 < 512:
                                    off = i0 - j0 + EB_OFF
                                    nc.vector.tensor_mul(
                                        p[:, 0:512], p[:, 0:512],
                                        ebs[sA][:, off:off + 512])
                                    nc.vector.tensor_mul(
                                        p[:, 512:1024], p[:, 512:1024],
                                        ebs[sB][:, off:off + 512])
                                cur.append((j, isl, p))
                            emit_av_batch(pend)
                            pend = cur
                        emit_av_batch(pend)
                        # evacuate oT psum (y + den) -> SBUF bf16, one cast
                        # per slot; frees the banks for the next half/pair
                        i0 = half * 1024
                        for t in range(2):
                            slot = sA if t == 0 else sB
                            last_evac = nc.vector.tensor_copy(
                                ysb[slot][:, i0:i0 + 1024], oTs[t][0:65, :])
                        if half == 0:
                            half0_evac = last_evac
                    while fi < len(fillers):
                        fillers[fi][1]()
                        fi += 1
                return half0_evac, last_evac

            # ================= emission schedule =================
            # Interleaved front: the preamble computes just enough QKV/V
            # (k01, q01 half-0, v chunks 0-3) for attention to start; the
            # rest of QKV and V are PE fillers inside attn0's j-loop so
            # the Scalar exp stream overlaps the projection matmuls.
            appool = tc.tile_pool(name="attn", bufs=1)
            apool[0] = appool.__enter__()
            sc = nc.enter_named_scope("qkv0", False)
            qkv_hh(2, 0, 0)
            qkv_hh(0, 0, 0)
            qkv_hh(2, 0, 1)
            vdir_jp(0)
            qkv_hh(0, 0, 1)
            vdir_jp(1)
            vdir_jp(2)
            nc.leave_named_scope("qkv0", sc[0], False)

            _ = attn_pair(0, [
                (5, lambda: vdir_jp(3)),
                (8, lambda: qkv_ip(2, 1)),
                (11, lambda: vdir_jp(4)),
                (14, lambda: vdir_jp(5)),
                (17, lambda: vdir_jp(6)),
                (20, late_dmas),
                (21, lambda: vdir_jp(7)),
                (24, lambda: qkv_ip(0, 1)),
                (29, lambda: stage_half(0, 0)),
                (30, lambda: qkv_ip(3, 0)),
                (34, lambda: qkv_ip(3, 1)),
                (38, lambda: qkv_ip(1, 0)),
            ])
            emit_a2a_pair(0)
            ev_half0, ev_last = attn_pair(1, [
                (6, lambda: qkv_ip(1, 1)),
                (30, lambda: stage_half(1, 0)),
            ])
            emit_a2a_pair(1)
            emit_ytf_pair(0, after=(ev_last,), gather_after=(ev_half0,))
            for ci in range(4):
                nm = norm_ytf(ci)
            for co in range(8):
                p1mm, p1cp = proj_pass1(co)
            emit_ytf_pair(1, after=(nm, p1cp), gather_after=(nm, p1cp))
            for ci in range(4, 8):
                norm_ytf(ci, after=(p1mm,))

            if dbg:
                for m in range(4):
                    t = dbg_tensor(f"dbg_qk{m}", [128, N], BF)
                    nc.sync.dma_start(t.ap()[:, :], qkp[m][:])
                t = dbg_tensor("dbg_vt0", [128, 2 * G * 65], BF)
                nc.sync.dma_start(t.ap()[:, :], vts[0][:])
                for s_ in range(4):
                    t = dbg_tensor(f"dbg_ysb{s_}", [65, N], BF)
                    nc.sync.dma_start(t.ap()[:, :], ysb[s_][:])
                for hh in range(2):
                    t = dbg_tensor(f"dbg_ytf{hh}", [128, 512], BF)
                    nc.sync.dma_start(t.ap()[:, :], ytf[hh][:])
                t = dbg_tensor("dbg_oacc0", [128, 512], F32)
                nc.sync.dma_start(t.ap()[:, :], ot_acc[0][:])

            sc = nc.enter_named_scope("proj", False)
            for co in range(8):
                proj_pass2(co)
            nc.leave_named_scope("proj", sc[0], False)
            appool.__exit__(None, None, None)
            ph1.__exit__(None, None, None)

    nc.compile()
    return nc


def prep_inputs(x, qkv_w, proj_w, proj_b, slopes):
    """Build the 8 per-core input maps (all host-side numpy)."""
    x = np.asarray(x, np.float32)
    qkv_w = np.asarray(qkv_w, np.float32)
    proj_w = np.asarray(proj_w, np.float32)
    proj_b = np.asarray(proj_b, np.float32)
    slopes = np.asarray(slopes, np.float32)

    # RoPE tables (transposed [d, n], d pairwise-interleaved, x2 head copies)
    inv = 1.0 / (10000.0 ** (np.arange(0, D, 2, dtype=np.float32) / D))
    fr = np.arange(N, dtype=np.float32)[:, None] * inv[None, :]   # [N, 32]
    sin_t, cos_t = np.sin(fr), np.cos(fr)
    ct64 = np.empty((64, N), np.float32)
    st64 = np.empty((64, N), np.float32)
    ct64[0::2] = cos_t.T
    ct64[1::2] = cos_t.T
    st64[0::2] = -sin_t.T
    st64[1::2] = sin_t.T
    ct = np.ascontiguousarray(np.vstack([ct64, ct64])).astype(NPBF)
    st = np.ascontiguousarray(np.vstack([st64, st64])).astype(NPBF)

    pos_p = np.arange(128, dtype=np.float64)[:, None]
    t_off = np.arange(EB_W, dtype=np.float64)[None, :] - EB_OFF
    dmin = np.minimum(pos_p - t_off, 0.0)  # j - i clipped

    in_maps = []
    for core in range(N_CORES):
        b = core // 4
        g = core % 4
        heads = GROUP_HEADS[g]
        rows = []
        for kind in range(2):  # q, k (d-permuted)
            for h in heads:
                base = kind * C + h * D
                rows.extend(base + p for p in D_PERM)
        for h in heads:        # v (natural d order)
            rows.extend(2 * C + h * D + d for d in range(D))
        wT_c = np.ascontiguousarray(qkv_w[rows, :].T)      # [1024, 768]

        eb_c = np.empty((G, 128, EB_W), np.float32)
        for s, h in enumerate(heads):
            eb_c[s] = np.exp(float(slopes[h]) * MAX_BIAS * dmin)

        # pwT rows (ci) ordered as the a2a outputs: ci = 2*slot + hh rows =
        # [rank 2hh (slot's head, 64 d), rank 2hh+1 (...)]
        pwT_c = np.empty((C, C), np.float32)
        for s in range(4):
            for rank in range(4):
                hh = GROUP_HEADS[rank][s]
                r = 256 * s + 64 * rank
                pwT_c[r:r + 64, :] = proj_w[:, hh * D:(hh + 1) * D].T
        pb_c = np.ascontiguousarray(proj_b.reshape(8, 128).T)

        # gather row offsets into ag_out [520, 1024]: my batch's rows start
        # at 260*b; rank r's block at 65*r, 64 y rows + 1 den row each.
        qoff_c = np.zeros((128, 3), np.uint32)
        p_ = np.arange(128)
        for hh in range(2):
            r = 2 * hh + p_ // 64
            qoff_c[:, hh] = 260 * b + 65 * r + p_ % 64
        qoff_c[0:4, 2] = 260 * b + 65 * np.arange(4) + 64

        wsel_c = np.zeros((4, 256), np.float32)
        for hh in range(2):
            wsel_c[2 * hh, 128 * hh:128 * hh + 64] = 1.0
            wsel_c[2 * hh + 1, 128 * hh + 64:128 * hh + 128] = 1.0

        in_maps.append({
            "xT": np.ascontiguousarray(x[b].T).astype(NPBF),
            "wT": wT_c.astype(NPBF),
            "ct": ct, "st": st,
            "ebias": eb_c.astype(NPBF),
            "pwT": pwT_c.astype(NPBF),
            "pb": pb_c,
            "qoff": qoff_c,
            "wselt": wsel_c.astype(NPBF),
        })
    return in_maps


_NC = None


def _get_nc():
    global _NC
    if _NC is None:
        _NC = build_program()
    return _NC


def run(inputs, trace=False, **kw):
    nc = _get_nc()
    in_maps = prep_inputs(**inputs)
    res = run_bass_kernel_spmd(nc, in_maps, core_ids=list(range(N_CORES)),
                               trace=trace, **kw)
    out = np.empty((B, N, C), np.float32)
    for core in range(N_CORES):
        b, g = core // 4, core % 4
        out[b, g * 512:(g + 1) * 512, :] = res.results[core]["out"].T
    return out, res


def kernel(**inputs) -> np.ndarray:
    out, _ = run(inputs, trace=False)
    return out
